# revision 12
# baseline (speedup 1.0000x reference)
"""Trainium2 Bass kernel for a basic RNN (concat -> linear -> tanh, + output proj).

Contract: kernel(**inputs) takes FULL unsharded inputs (numpy), returns the
same structure as the reference: (output [B,S,D_OUT] f32, predict [B,F,D_OUT] f32).

Strategy: data-parallel over batch (64 -> 8 cores x 8 rows). Weights replicated.
All matmul operands bf16 (fp32 PSUM accumulation); per-step h@Whh uses 4-way
PE column tiling; h is kept hidden-major via row-tiled PE transposes.
"""

import sys
from contextlib import ExitStack

try:
    import concourse.bass as bass  # noqa
except ImportError:
    sys.path.insert(0, "/opt/trn_rl_repo")

import numpy as np
import ml_dtypes

import concourse.bass as bass
import concourse.tile as tile
from concourse import bacc, mybir
from concourse.bass import ts, ds
from concourse.bass_utils import run_bass_kernel_spmd

BF16 = mybir.dt.bfloat16
F32 = mybir.dt.float32
NPBF16 = ml_dtypes.bfloat16

B, D_IN, D_H, D_OUT = 64, 1024, 2048, 1024
NCORES = 8
BL = B // NCORES          # 8 local batch rows per core
KI = D_IN // 128          # 8 k-tiles for input features
KH = D_H // 128           # 16 k-tiles for hidden
NJ = 4                    # column-tiling groups (hidden 2048 = 4 x 512)
GS = 16                   # steps per group (= 128 tokens per output tile)
Tanh = mybir.ActivationFunctionType.Tanh
Copy = mybir.ActivationFunctionType.Copy


def build(S, FUT, static_groups=True):
    """Build and compile the per-core Bass program. Returns compiled nc."""
    assert S % GS == 0
    n_groups = S // GS
    TOK = S * BL

    nc = bacc.Bacc("TRN2", target_bir_lowering=False, debug=False, num_devices=NCORES)

    xT = nc.dram_tensor("xT", [D_IN, TOK], BF16, kind="ExternalInput").ap()
    Wx_d = nc.dram_tensor("Wx", [D_IN, D_H], BF16, kind="ExternalInput").ap()
    Whh_d = nc.dram_tensor("Whh", [D_H, D_H], BF16, kind="ExternalInput").ap()
    Wo_d = nc.dram_tensor("Wo", [D_H, D_OUT], BF16, kind="ExternalInput").ap()
    bh_d = nc.dram_tensor("bh", [1, D_H], BF16, kind="ExternalInput").ap()
    bo_d = nc.dram_tensor("bo", [1, D_OUT], BF16, kind="ExternalInput").ap()
    idr_d = nc.dram_tensor("idr", [128, BL], BF16, kind="ExternalInput").ap()
    idm_d = nc.dram_tensor("idm", [BL, 32], BF16, kind="ExternalInput").ap()
    onesm_d = nc.dram_tensor("onesm", [1, 32], BF16, kind="ExternalInput").ap()
    ones_d = nc.dram_tensor("ones", [1, 128], BF16, kind="ExternalInput").ap()
    out_d = nc.dram_tensor("out", [TOK, D_OUT], F32, kind="ExternalOutput").ap()
    if FUT > 0:
        pred_d = nc.dram_tensor("pred", [FUT * BL, D_OUT], F32, kind="ExternalOutput").ap()
    P_d = nc.dram_tensor("P", [TOK, D_H], BF16).ap()

    xT_r = xT.rearrange("(k p) t -> p k t", p=128)      # [128, KI, TOK]

    with tile.TileContext(nc) as tc, ExitStack() as stk:
        cpool = stk.enter_context(tc.tile_pool(name="consts", bufs=1))
        wx = cpool.tile([128, KI, D_H], BF16, tag="wx")
        whh = cpool.tile([128, KH, D_H], BF16, tag="whh")
        wo = cpool.tile([128, KH, D_OUT], BF16, tag="wo")
        bh = cpool.tile([1, D_H], BF16, tag="bh")
        bo = cpool.tile([1, D_OUT], BF16, tag="bo")
        idr = cpool.tile([128, BL], BF16, tag="idr")
        idm = cpool.tile([BL, 32], BF16, tag="idm")
        onesm = cpool.tile([1, 32], BF16, tag="onesm")
        ones = cpool.tile([1, 128], BF16, tag="ones")
        # hidden-major h ring: [128, slot, k, b]
        ring = cpool.tile([128, KH, GS, BL], BF16, tag="ring")
        ring_f = ring.rearrange("p k t b -> p (k t b)")
        inpT = cpool.tile([128, KI, BL], BF16, tag="inpT")
        inpT_f = inpT.rearrange("p k b -> p (k b)")

        # --- setup: weights + consts in, ring zeroed ---
        for k in range(KI):
            nc.sync.dma_start(wx[:, k, :], Wx_d.rearrange("(k p) n -> p k n", p=128)[:, k, :])
        for k in range(KH):
            nc.sync.dma_start(whh[:, k, :], Whh_d.rearrange("(k p) n -> p k n", p=128)[:, k, :])
        for k in range(KH):
            nc.sync.dma_start(wo[:, k, :], Wo_d.rearrange("(k p) n -> p k n", p=128)[:, k, :])
        nc.sync.dma_start(bh[:], bh_d[:])
        nc.sync.dma_start(bo[:], bo_d[:])
        nc.sync.dma_start(idr[:], idr_d[:])
        nc.sync.dma_start(idm[:], idm_d[:])
        nc.sync.dma_start(onesm[:], onesm_d[:])
        nc.sync.dma_start(ones[:], ones_d[:])
        nc.gpsimd.memset(ring_f[:, :], 0.0)

        xt_pool = stk.enter_context(tc.tile_pool(name="xt", bufs=3))
        ps_pool = stk.enter_context(tc.tile_pool(name="psum", bufs=2, space="PSUM"))
        pst_pool = stk.enter_context(tc.tile_pool(name="pstage", bufs=2))
        pt_pool = stk.enter_context(tc.tile_pool(name="pt", bufs=4))
        hp_pool = stk.enter_context(tc.tile_pool(name="hpad", bufs=3))
        os_pool = stk.enter_context(tc.tile_pool(name="ostage", bufs=2))

        # --- phase 1: P = x @ Wx + bh  (token-tiles of 128) ---
        for g in range(TOK // 128):
            xt = xt_pool.tile([128, KI, 128], BF16, tag="xt")
            nc.sync.dma_start(xt[:], xT_r[:, :, ts(g, 128)])
            pstage = pst_pool.tile([128, D_H], BF16, tag="pstage")
            for n in range(NJ):
                zp = ps_pool.tile([128, 512], F32, tag="z")
                for k in range(KI):
                    nc.tensor.matmul(zp[:], xt[:, k, :], wx[:, k, ts(n, 512)],
                                     start=(k == 0), stop=False)
                nc.tensor.matmul(zp[:], ones[0:1, :], bh[0:1, ts(n, 512)],
                                 start=False, stop=True)
                nc.scalar.activation(pstage[:, ts(n, 512)], zp[:], Copy)
            nc.sync.dma_start(P_d[ts(g, 128), :], pstage[:])

        # --- phase 2: recurrence + inline output projection ---
        def _accum_p(g, tau):
            # P_t + bh accumulation: first writes of the step's z (start=True)
            pt = pt_pool.tile([BL, D_H], BF16, tag="pt")
            nc.sync.dma_start(pt[:], P_d[ds(g * 128 + tau * BL, BL), :])
            zp = ps_pool.tile([128, 512], F32, tag="z")
            for j in range(NJ):
                nc.tensor.matmul(zp[32*j:32*j+BL, :],
                                 idr[0:BL, :], pt[:, ts(j, 512)],
                                 start=True, stop=False,
                                 tile_position=(0, 32 * j),
                                 skip_group_check=True)
            return zp

        def group_body(g, zp0=None):
            zp = zp0 if zp0 is not None else _accum_p(g, 0)
            for tau in range(GS):
                prev = (tau - 1) % GS
                for k in range(KH):
                    for j in range(NJ):
                        nc.tensor.matmul(zp[32*j:32*j+BL, :],
                                         ring[:, k, prev, :],
                                         whh[:, k, ts(j, 512)],
                                         start=False, stop=(k == KH - 1),
                                         tile_position=(0, 32 * j),
                                         skip_group_check=True)
                # prefetch next step's z into the tanh-wait bubble
                zp_next = _accum_p(g, tau + 1) if tau + 1 < GS else None
                hp = hp_pool.tile([128, 512], BF16, tag="hp")
                for j in range(NJ):
                    nc.scalar.activation(hp[32*j:32*j+BL, :], zp[32*j:32*j+BL, :], Tanh)
                tp = ps_pool.tile([128, KH * BL], BF16, tag="t")
                for c in range(KH):
                    j = c // 4
                    nc.tensor.matmul(tp[:, ts(c, BL)],
                                     hp[32*j:32*j+BL, ts(c % 4, 128)],
                                     idr[32*j:32*j+BL, 0:BL],
                                     is_transpose=True, tile_position=(32 * j, 0))
                nc.vector.tensor_copy(ring[:, :, tau, :], tp.rearrange("p (k b) -> p k b", b=BL))
                zp = zp_next
            # output projection for this group's 128 tokens
            ost = os_pool.tile([128, D_OUT], F32, tag="ostage")
            for n in range(2):
                op = ps_pool.tile([128, 512], F32, tag="ob")
                for k in range(KH):
                    nc.tensor.matmul(op[:], ring[:, k, :, :], wo[:, k, ts(n, 512)],
                                     start=(k == 0), stop=False)
                nc.tensor.matmul(op[:], ones[0:1, :], bo[0:1, ts(n, 512)],
                                 start=False, stop=True)
                nc.vector.tensor_copy(ost[:, ts(n, 512)], op[:])
            nc.sync.dma_start(out_d[ds(g * 128, 128), :], ost[:])

        if static_groups:
            for g in range(n_groups):
                group_body(g)
        else:
            with tc.For_i(0, n_groups, 1) as g:
                group_body(g)

        # --- phase 3: future prediction, feeding outputs back ---
        if FUT > 0:
            nc.sync.dma_start(inpT[:], xT_r[:, :, TOK - BL:TOK])
            for f in range(FUT):
                slot = f % GS
                prev = (f - 1) % GS  # f=0 -> 15, last slot of main phase
                zp = ps_pool.tile([128, 512], F32, tag="z")
                for k in range(KI + KH):
                    lhs = inpT[:, k, :] if k < KI else ring[:, k - KI, prev, :]
                    for j in range(NJ):
                        rhs = wx[:, k, ts(j, 512)] if k < KI else whh[:, k - KI, ts(j, 512)]
                        nc.tensor.matmul(zp[32*j:32*j+BL, :], lhs, rhs,
                                         start=(k == 0), stop=False,
                                         tile_position=(0, 32 * j),
                                         skip_group_check=True)
                for j in range(NJ):
                    nc.tensor.matmul(zp[32*j:32*j+BL, :],
                                     ones[0:1, 0:BL], bh[0:1, ts(j, 512)],
                                     start=False, stop=True,
                                     tile_position=(0, 32 * j),
                                     skip_group_check=True)
                hp = hp_pool.tile([128, 512], BF16, tag="hp")
                for j in range(NJ):
                    nc.scalar.activation(hp[32*j:32*j+BL, :], zp[32*j:32*j+BL, :], Tanh)
                tp = ps_pool.tile([128, KH * BL], BF16, tag="t")
                for c in range(KH):
                    j = c // 4
                    nc.tensor.matmul(tp[:, ts(c, BL)],
                                     hp[32*j:32*j+BL, ts(c % 4, 128)],
                                     idr[32*j:32*j+BL, 0:BL],
                                     is_transpose=True, tile_position=(32 * j, 0))
                nc.vector.tensor_copy(ring[:, :, slot, :], tp.rearrange("p (k b) -> p k b", b=BL))
                # out_t = h @ Wo + bo  (2 n-chunks in col groups 0 and 1)
                ot = ps_pool.tile([128, 512], F32, tag="ob")
                for n in range(2):
                    for k in range(KH):
                        nc.tensor.matmul(ot[32*n:32*n+BL, :],
                                         ring[:, k, slot, :], wo[:, k, ts(n, 512)],
                                         start=(k == 0), stop=False,
                                         tile_position=(0, 32 * n),
                                         skip_group_check=True)
                    nc.tensor.matmul(ot[32*n:32*n+BL, :],
                                     ones[0:1, 0:BL], bo[0:1, ts(n, 512)],
                                     start=False, stop=True,
                                     tile_position=(0, 32 * n),
                                     skip_group_check=True)
                prs = os_pool.tile([128, D_OUT], F32, tag="prs")
                for n in range(2):
                    nc.vector.tensor_copy(prs[32*n:32*n+BL, ts(n, 512)],
                                          ot[32*n:32*n+BL, :])
                    nc.sync.dma_start(pred_d[ds(f * BL, BL), ts(n, 512)],
                                      prs[32*n:32*n+BL, ts(n, 512)])
                if f < FUT - 1:
                    ob = hp_pool.tile([128, 512], BF16, tag="obf")
                    for n in range(2):
                        nc.scalar.activation(ob[32*n:32*n+BL, :], ot[32*n:32*n+BL, :], Copy)
                    ip = ps_pool.tile([128, KI * BL], BF16, tag="it")
                    for c in range(KI):
                        n = c // 4
                        nc.tensor.matmul(ip[:, ts(c, BL)],
                                         ob[32*n:32*n+BL, ts(c % 4, 128)],
                                         idr[32*n:32*n+BL, 0:BL],
                                         is_transpose=True, tile_position=(32 * n, 0))
                    nc.vector.tensor_copy(inpT_f[:, 0:KI * BL], ip[:])

    nc.compile()
    return nc


def _prep_core_inputs(x_c, Wx, Whh, Wo, bh, bo, idr, ones, idm, onesm):
    S = x_c.shape[1]
    xT = np.ascontiguousarray(x_c.transpose(2, 1, 0).reshape(D_IN, S * BL)).astype(NPBF16)
    return {"xT": xT, "Wx": Wx, "Whh": Whh, "Wo": Wo, "bh": bh, "bo": bo,
            "idr": idr, "ones": ones, "idm": idm, "onesm": onesm}


def run(nc, x, Wh, bh, bo_, Wo, S, FUT, trace=False):
    Wx = np.ascontiguousarray(Wh[:D_IN]).astype(NPBF16)
    Whh = np.ascontiguousarray(Wh[D_IN:]).astype(NPBF16)
    Wob = Wo.astype(NPBF16)
    bhb = bh.reshape(1, D_H).astype(NPBF16)
    bob = bo_.reshape(1, D_OUT).astype(NPBF16)
    idr = np.zeros((128, BL), dtype=NPBF16)
    for j in range(4):
        for b in range(BL):
            idr[32 * j + b, b] = 1.0
    ones = np.ones((1, 128), dtype=NPBF16)
    idm = np.zeros((BL, 32), dtype=NPBF16)
    for b in range(BL):
        idm[b, b] = 1.0
    onesm = np.zeros((1, 32), dtype=NPBF16)
    onesm[0, 0:BL] = 1.0

    in_maps = []
    for c in range(NCORES):
        x_c = x[c * BL:(c + 1) * BL]
        in_maps.append(_prep_core_inputs(x_c, Wx, Whh, Wob, bhb, bob, idr, ones, idm, onesm))
    res = run_bass_kernel_spmd(nc, in_maps, list(range(NCORES)), trace=trace)

    out = np.empty((B, S, D_OUT), dtype=np.float32)
    pred = np.zeros((B, max(FUT, 0), D_OUT), dtype=np.float32)
    for c in range(NCORES):
        r = res.results[c]
        out[c * BL:(c + 1) * BL] = r["out"].reshape(S, BL, D_OUT).transpose(1, 0, 2)
        if FUT > 0:
            pred[c * BL:(c + 1) * BL] = r["pred"].reshape(FUT, BL, D_OUT).transpose(1, 0, 2)
    return out, pred, res


_CACHE = {}


def kernel(x, Wh, bh, Wo, bo, future):
    x = np.asarray(x, dtype=np.float32)
    Wh = np.asarray(Wh, dtype=np.float32)
    bh_a = np.asarray(bh, dtype=np.float32)
    Wo_a = np.asarray(Wo, dtype=np.float32)
    bo_a = np.asarray(bo, dtype=np.float32)
    FUT = int(future)
    S = x.shape[1]
    key = (S, FUT)
    if key not in _CACHE:
        _CACHE[key] = build(S, FUT)
    nc = _CACHE[key]
    out, pred, _ = run(nc, x, Wh, bh_a, bo_a, Wo_a, S, FUT)
    return out, pred


# revision 16
# speedup vs baseline: 1.3516x; 1.3516x over previous
"""Trainium2 Bass kernel for a basic RNN (concat -> linear -> tanh, + output proj).

Contract: kernel(**inputs) takes FULL unsharded inputs (numpy), returns the
same structure as the reference: (output [B,S,D_OUT] f32, predict [B,F,D_OUT] f32).

Strategy: data-parallel over batch (64 -> 8 cores x 8 rows). Weights replicated.
All matmul operands bf16 (fp32 PSUM accumulation); per-step h@Whh uses 4-way
PE column tiling; h is kept hidden-major via row-tiled PE transposes.
"""

import sys
from contextlib import ExitStack

try:
    import concourse.bass as bass  # noqa
except ImportError:
    sys.path.insert(0, "/opt/trn_rl_repo")

import numpy as np
import ml_dtypes

import concourse.bass as bass
import concourse.tile as tile
from concourse import bacc, mybir
from concourse.bass import ts, ds
from concourse.bass_utils import run_bass_kernel_spmd

BF16 = mybir.dt.bfloat16
F32 = mybir.dt.float32
NPBF16 = ml_dtypes.bfloat16

B, D_IN, D_H, D_OUT = 64, 1024, 2048, 1024
NCORES = 8
BL = B // NCORES          # 8 local batch rows per core
KI = D_IN // 128          # 8 k-tiles for input features
KH = D_H // 128           # 16 k-tiles for hidden
NJ = 4                    # column-tiling groups (hidden 2048 = 4 x 512)
GS = 16                   # steps per group (= 128 tokens per output tile)
Tanh = mybir.ActivationFunctionType.Tanh
Copy = mybir.ActivationFunctionType.Copy


def build(S, FUT, static_groups=True, repeat=1):
    """Build and compile the per-core Bass program. Returns compiled nc."""
    assert S % GS == 0
    n_groups = S // GS
    TOK = S * BL

    nc = bacc.Bacc("TRN2", target_bir_lowering=False, debug=False, num_devices=NCORES)

    xT = nc.dram_tensor("xT", [D_IN, TOK], BF16, kind="ExternalInput").ap()
    Wx_d = nc.dram_tensor("Wx", [D_IN, D_H], BF16, kind="ExternalInput").ap()
    Whh_d = nc.dram_tensor("Whh", [D_H, D_H], BF16, kind="ExternalInput").ap()
    Wo_d = nc.dram_tensor("Wo", [D_H, D_OUT], BF16, kind="ExternalInput").ap()
    bh_d = nc.dram_tensor("bh", [1, D_H], BF16, kind="ExternalInput").ap()
    bo_d = nc.dram_tensor("bo", [1, D_OUT], BF16, kind="ExternalInput").ap()
    idr_d = nc.dram_tensor("idr", [128, BL], BF16, kind="ExternalInput").ap()
    idm_d = nc.dram_tensor("idm", [BL, 32], BF16, kind="ExternalInput").ap()
    onesm_d = nc.dram_tensor("onesm", [1, 32], BF16, kind="ExternalInput").ap()
    ones_d = nc.dram_tensor("ones", [1, 128], BF16, kind="ExternalInput").ap()
    out_d = nc.dram_tensor("out", [TOK, D_OUT], F32, kind="ExternalOutput").ap()
    if FUT > 0:
        pred_d = nc.dram_tensor("pred", [FUT * BL, D_OUT], F32, kind="ExternalOutput").ap()
    P_d = nc.dram_tensor("P", [TOK, D_H], BF16).ap()

    xT_r = xT.rearrange("(k p) t -> p k t", p=128)      # [128, KI, TOK]

    with tile.TileContext(nc) as tc, ExitStack() as stk:
        cpool = stk.enter_context(tc.tile_pool(name="consts", bufs=1))
        wx = cpool.tile([128, KI, D_H], BF16, tag="wx")
        whh = cpool.tile([128, KH, D_H], BF16, tag="whh")
        wo = cpool.tile([128, KH, D_OUT], BF16, tag="wo")
        bh = cpool.tile([1, D_H], BF16, tag="bh")
        bo = cpool.tile([1, D_OUT], BF16, tag="bo")
        idr = cpool.tile([128, BL], BF16, tag="idr")
        idm = cpool.tile([BL, 32], BF16, tag="idm")
        onesm = cpool.tile([1, 32], BF16, tag="onesm")
        ones = cpool.tile([1, 128], BF16, tag="ones")
        # hidden-major h ring: [128, slot, k, b]
        ring = cpool.tile([128, KH, GS, BL], BF16, tag="ring")
        ring_f = ring.rearrange("p k t b -> p (k t b)")
        inpT = cpool.tile([128, KI, BL], BF16, tag="inpT")
        inpT_f = inpT.rearrange("p k b -> p (k b)")

        # --- setup: weights + consts in, ring zeroed ---
        for k in range(KI):
            nc.sync.dma_start(wx[:, k, :], Wx_d.rearrange("(k p) n -> p k n", p=128)[:, k, :])
        for k in range(KH):
            nc.sync.dma_start(whh[:, k, :], Whh_d.rearrange("(k p) n -> p k n", p=128)[:, k, :])
        for k in range(KH):
            nc.sync.dma_start(wo[:, k, :], Wo_d.rearrange("(k p) n -> p k n", p=128)[:, k, :])
        nc.sync.dma_start(bh[:], bh_d[:])
        nc.sync.dma_start(bo[:], bo_d[:])
        nc.sync.dma_start(idr[:], idr_d[:])
        nc.sync.dma_start(idm[:], idm_d[:])
        nc.sync.dma_start(onesm[:], onesm_d[:])
        nc.sync.dma_start(ones[:], ones_d[:])
        nc.gpsimd.memset(ring_f[:, :], 0.0)

        xt_pool = stk.enter_context(tc.tile_pool(name="xt", bufs=3))
        ps_pool = stk.enter_context(tc.tile_pool(name="psum", bufs=2, space="PSUM"))
        pst_pool = stk.enter_context(tc.tile_pool(name="pstage", bufs=2))
        pt_pool = stk.enter_context(tc.tile_pool(name="pt", bufs=4))
        hp_pool = stk.enter_context(tc.tile_pool(name="hpad", bufs=3))
        os_pool = stk.enter_context(tc.tile_pool(name="ostage", bufs=2))

        # --- phase 1..3, optionally repeated for delta timing ---
        for _rep in range(repeat):
         # --- phase 1: P = x @ Wx + bh  (token-tiles of 128) ---
         for g in range(TOK // 128):
            xt = xt_pool.tile([128, KI, 128], BF16, tag="xt")
            nc.sync.dma_start(xt[:], xT_r[:, :, ts(g, 128)])
            pstage = pst_pool.tile([128, D_H], BF16, tag="pstage")
            for n in range(NJ):
                zp = ps_pool.tile([128, 512], F32, tag="z")
                for k in range(KI):
                    nc.tensor.matmul(zp[:], xt[:, k, :], wx[:, k, ts(n, 512)],
                                     start=(k == 0), stop=False)
                nc.tensor.matmul(zp[:], ones[0:1, :], bh[0:1, ts(n, 512)],
                                 start=False, stop=True)
                nc.scalar.activation(pstage[:, ts(n, 512)], zp[:], Copy)
            nc.sync.dma_start(P_d[ts(g, 128), :], pstage[:])
         _run_phases23()

        # --- phase 2: recurrence + inline output projection ---
        def _accum_p(g, tau):
            # P_t + bh accumulation: first writes of the step's z (start=True)
            pt = pt_pool.tile([BL, D_H], BF16, tag="pt")
            nc.sync.dma_start(pt[:], P_d[ds(g * 128 + tau * BL, BL), :])
            zp = ps_pool.tile([128, 512], F32, tag="z")
            for j in range(NJ):
                nc.tensor.matmul(zp[32*j:32*j+BL, :],
                                 idr[0:BL, :], pt[:, ts(j, 512)],
                                 start=True, stop=False,
                                 tile_position=(0, 32 * j),
                                 skip_group_check=True)
            return zp

        def group_body(g, zp0=None):
            zp = zp0 if zp0 is not None else _accum_p(g, 0)
            for tau in range(GS):
                prev = (tau - 1) % GS
                for k in range(KH):
                    for j in range(NJ):
                        nc.tensor.matmul(zp[32*j:32*j+BL, :],
                                         ring[:, k, prev, :],
                                         whh[:, k, ts(j, 512)],
                                         start=False, stop=(k == KH - 1),
                                         tile_position=(0, 32 * j),
                                         skip_group_check=True)
                # prefetch next step's z into the tanh-wait bubble
                zp_next = _accum_p(g, tau + 1) if tau + 1 < GS else None
                hp = hp_pool.tile([128, 512], BF16, tag="hp")
                for j in range(NJ):
                    nc.scalar.activation(hp[32*j:32*j+BL, :], zp[32*j:32*j+BL, :], Tanh)
                tp = ps_pool.tile([128, KH * BL], BF16, tag="t")
                for c in range(KH):
                    j = c // 4
                    nc.tensor.matmul(tp[:, ts(c, BL)],
                                     hp[32*j:32*j+BL, ts(c % 4, 128)],
                                     idr[32*j:32*j+BL, 0:BL],
                                     is_transpose=True, tile_position=(32 * j, 0))
                tpr = tp.rearrange("p (k b) -> p k b", b=BL)
                for j in range(NJ):
                    nc.vector.tensor_copy(ring[:, 4*j:4*j+4, tau, :], tpr[:, 4*j:4*j+4, :])
                zp = zp_next
            # output projection for this group's 128 tokens
            ost = os_pool.tile([128, D_OUT], F32, tag="ostage")
            for n in range(2):
                op = ps_pool.tile([128, 512], F32, tag="ob")
                for k in range(KH):
                    nc.tensor.matmul(op[:], ring[:, k, :, :], wo[:, k, ts(n, 512)],
                                     start=(k == 0), stop=False)
                nc.tensor.matmul(op[:], ones[0:1, :], bo[0:1, ts(n, 512)],
                                 start=False, stop=True)
                nc.vector.tensor_copy(ost[:, ts(n, 512)], op[:])
            nc.sync.dma_start(out_d[ds(g * 128, 128), :], ost[:])

        def _run_phases23():
         if static_groups:
            for g in range(n_groups):
                group_body(g)
         else:
            with tc.For_i(0, n_groups, 1) as g:
                group_body(g)

         # --- phase 3: future prediction, feeding outputs back ---
         if FUT > 0:
            nc.sync.dma_start(inpT[:], xT_r[:, :, TOK - BL:TOK])
            for f in range(FUT):
                slot = f % GS
                prev = (f - 1) % GS  # f=0 -> 15, last slot of main phase
                zp = ps_pool.tile([128, 512], F32, tag="z")
                for k in range(KI + KH):
                    lhs = inpT[:, k, :] if k < KI else ring[:, k - KI, prev, :]
                    for j in range(NJ):
                        rhs = wx[:, k, ts(j, 512)] if k < KI else whh[:, k - KI, ts(j, 512)]
                        nc.tensor.matmul(zp[32*j:32*j+BL, :], lhs, rhs,
                                         start=(k == 0), stop=False,
                                         tile_position=(0, 32 * j),
                                         skip_group_check=True)
                for j in range(NJ):
                    nc.tensor.matmul(zp[32*j:32*j+BL, :],
                                     ones[0:1, 0:BL], bh[0:1, ts(j, 512)],
                                     start=False, stop=True,
                                     tile_position=(0, 32 * j),
                                     skip_group_check=True)
                hp = hp_pool.tile([128, 512], BF16, tag="hp")
                for j in range(NJ):
                    nc.scalar.activation(hp[32*j:32*j+BL, :], zp[32*j:32*j+BL, :], Tanh)
                tp = ps_pool.tile([128, KH * BL], BF16, tag="t")
                for c in range(KH):
                    j = c // 4
                    nc.tensor.matmul(tp[:, ts(c, BL)],
                                     hp[32*j:32*j+BL, ts(c % 4, 128)],
                                     idr[32*j:32*j+BL, 0:BL],
                                     is_transpose=True, tile_position=(32 * j, 0))
                tpr = tp.rearrange("p (k b) -> p k b", b=BL)
                for j in range(NJ):
                    nc.vector.tensor_copy(ring[:, 4*j:4*j+4, slot, :], tpr[:, 4*j:4*j+4, :])
                # out_t = h @ Wo + bo  (2 n-chunks in col groups 0 and 1)
                ot = ps_pool.tile([128, 512], F32, tag="ob")
                for n in range(2):
                    for k in range(KH):
                        nc.tensor.matmul(ot[32*n:32*n+BL, :],
                                         ring[:, k, slot, :], wo[:, k, ts(n, 512)],
                                         start=(k == 0), stop=False,
                                         tile_position=(0, 32 * n),
                                         skip_group_check=True)
                    nc.tensor.matmul(ot[32*n:32*n+BL, :],
                                     ones[0:1, 0:BL], bo[0:1, ts(n, 512)],
                                     start=False, stop=True,
                                     tile_position=(0, 32 * n),
                                     skip_group_check=True)
                prs = os_pool.tile([128, D_OUT], F32, tag="prs")
                for n in range(2):
                    nc.vector.tensor_copy(prs[32*n:32*n+BL, ts(n, 512)],
                                          ot[32*n:32*n+BL, :])
                    nc.sync.dma_start(pred_d[ds(f * BL, BL), ts(n, 512)],
                                      prs[32*n:32*n+BL, ts(n, 512)])
                if f < FUT - 1:
                    ob = hp_pool.tile([128, 512], BF16, tag="obf")
                    for n in range(2):
                        nc.scalar.activation(ob[32*n:32*n+BL, :], ot[32*n:32*n+BL, :], Copy)
                    ip = ps_pool.tile([128, KI * BL], BF16, tag="it")
                    for c in range(KI):
                        n = c // 4
                        nc.tensor.matmul(ip[:, ts(c, BL)],
                                         ob[32*n:32*n+BL, ts(c % 4, 128)],
                                         idr[32*n:32*n+BL, 0:BL],
                                         is_transpose=True, tile_position=(32 * n, 0))
                    nc.vector.tensor_copy(inpT_f[:, 0:KI * BL], ip[:])

    nc.compile()
    return nc


def _prep_core_inputs(x_c, Wx, Whh, Wo, bh, bo, idr, ones, idm, onesm):
    S = x_c.shape[1]
    xT = np.ascontiguousarray(x_c.transpose(2, 1, 0).reshape(D_IN, S * BL)).astype(NPBF16)
    return {"xT": xT, "Wx": Wx, "Whh": Whh, "Wo": Wo, "bh": bh, "bo": bo,
            "idr": idr, "ones": ones, "idm": idm, "onesm": onesm}


def run(nc, x, Wh, bh, bo_, Wo, S, FUT, trace=False):
    Wx = np.ascontiguousarray(Wh[:D_IN]).astype(NPBF16)
    Whh = np.ascontiguousarray(Wh[D_IN:]).astype(NPBF16)
    Wob = Wo.astype(NPBF16)
    bhb = bh.reshape(1, D_H).astype(NPBF16)
    bob = bo_.reshape(1, D_OUT).astype(NPBF16)
    idr = np.zeros((128, BL), dtype=NPBF16)
    for j in range(4):
        for b in range(BL):
            idr[32 * j + b, b] = 1.0
    ones = np.ones((1, 128), dtype=NPBF16)
    idm = np.zeros((BL, 32), dtype=NPBF16)
    for b in range(BL):
        idm[b, b] = 1.0
    onesm = np.zeros((1, 32), dtype=NPBF16)
    onesm[0, 0:BL] = 1.0

    in_maps = []
    for c in range(NCORES):
        x_c = x[c * BL:(c + 1) * BL]
        in_maps.append(_prep_core_inputs(x_c, Wx, Whh, Wob, bhb, bob, idr, ones, idm, onesm))
    res = run_bass_kernel_spmd(nc, in_maps, list(range(NCORES)), trace=trace)

    out = np.empty((B, S, D_OUT), dtype=np.float32)
    pred = np.zeros((B, max(FUT, 0), D_OUT), dtype=np.float32)
    for c in range(NCORES):
        r = res.results[c]
        out[c * BL:(c + 1) * BL] = r["out"].reshape(S, BL, D_OUT).transpose(1, 0, 2)
        if FUT > 0:
            pred[c * BL:(c + 1) * BL] = r["pred"].reshape(FUT, BL, D_OUT).transpose(1, 0, 2)
    return out, pred, res


_CACHE = {}


def kernel(x, Wh, bh, Wo, bo, future):
    x = np.asarray(x, dtype=np.float32)
    Wh = np.asarray(Wh, dtype=np.float32)
    bh_a = np.asarray(bh, dtype=np.float32)
    Wo_a = np.asarray(Wo, dtype=np.float32)
    bo_a = np.asarray(bo, dtype=np.float32)
    FUT = int(future)
    S = x.shape[1]
    key = (S, FUT)
    if key not in _CACHE:
        _CACHE[key] = build(S, FUT)
    nc = _CACHE[key]
    out, pred, _ = run(nc, x, Wh, bh_a, bo_a, Wo_a, S, FUT)
    return out, pred
